# revision 22
# baseline (speedup 1.0000x reference)
"""DAS beamforming via GPSIMD ap_gather, u8-quantized alpha.

Layout (per NeuronCore, 8192-pixel shard):
  - 16 passes k; gpsimd core g (partitions 16g..16g+15) handles detector
    d = g*16 + k; lane j within a core carries batch b = j%4 (4 replicas).
  - pixel stream split in 2 halves of 4096 (PSUM capacity); within a half,
    stream position i corresponds to pixel h*4096 + i, and the ap_gather
    wrapped-index layout stores k0 for stream position i at
    idx[16g + i%16, i//16] (host prepares this wrap).
  - tables: pairs[c, t, 0:2] = [S(d,t,b), S(d,t+1,b)] fp16, built per pass
    by a PE replication matmul (rep^T @ series[32 x 2048] with a paired
    access pattern) since SBUF partitions cannot be broadcast by DMA.
  - ap_gather -> G[128, 4096, 2]; DVE lerp L = G0 + alpha*(G1-G0);
  - PE: psum[8, 512-chunks] += w2_k^T @ L, accumulating over all 16 passes
    (rows 0-3 = half 0 batches, rows 4-7 = half 1).
"""
import numpy as np

import concourse.bass as bass
import concourse.tile as tile
from concourse import bacc, mybir

N_DET, N_T, NY, NX, B = 128, 2048, 256, 256, 4
P_TOTAL = NY * NX
N_CORES = 8
PX_PER_CORE = P_TOTAL // N_CORES
HALF = PX_PER_CORE // 2            # 4096
N_PASS = 16
N_G = 8                            # gpsimd cores
F32 = mybir.dt.float32
F16 = mybir.dt.float16
I16 = mybir.dt.int16
U8 = mybir.dt.uint8


def _build_kernel():
    nc = bacc.Bacc("TRN2", target_bir_lowering=False, debug=False)

    # sbt3[m = 4g+b, k, t] = sino[b, 0, g*16+k, t]  (fp16 series)
    sbt3 = nc.dram_tensor("sbt3", [32, N_PASS * N_T], F16, kind="ExternalInput")
    # k0w[k, h, lane, s]: wrapped fp-floor indices (int16)
    k0w = nc.dram_tensor("k0w", [N_PASS * 2, 128 * (HALF // 16)], I16,
                         kind="ExternalInput")
    # alg[k, h, g, i] = alpha[pixel h*4096+i, det g*16+k]  (fp16)
    alg = nc.dram_tensor("alg", [N_PASS * 2, N_G * HALF], U8,
                         kind="ExternalInput")
    # w2[lane, (k*2+h)*8 + h*4 + bo] = apod_n[det] * (lane%4 == bo); the
    # other half's 4 columns in each 8-col group are zero so a full [8,512]
    # psum write accumulates += 0 on the other half's rows.
    w2 = nc.dram_tensor("w2", [128, N_PASS * 32], F16, kind="ExternalInput")
    # rep[m, lane] = 1 if lane's (g,b) == m  (fp16 replication matrix)
    repm = nc.dram_tensor("repm", [32, 128], F16, kind="ExternalInput")
    outd = nc.dram_tensor("out", [8, HALF], F32, kind="ExternalOutput")

    S_IDX = HALF // 16  # idx cols per half

    with tile.TileContext(nc) as tc:
        with (
            tc.tile_pool(name="const", bufs=1) as cpool,
            tc.tile_pool(name="ser", bufs=2) as ser,
            tc.tile_pool(name="tab", bufs=2) as tab,
            tc.tile_pool(name="tps", bufs=2, space="PSUM") as tps,
            tc.tile_pool(name="io", bufs=4) as io,
            tc.tile_pool(name="gat", bufs=2) as gat,
            tc.tile_pool(name="lrp", bufs=3) as lrp,
            tc.tile_pool(name="scp", bufs=4, space="PSUM") as scp,
            tc.tile_pool(name="oc", bufs=1) as oc,
        ):
            w2_tl = cpool.tile([128, N_PASS * 32], F16)
            nc.sync.dma_start(out=w2_tl[:], in_=w2.ap())
            rep_tl = cpool.tile([32, 128], F16)
            nc.sync.dma_start(out=rep_tl[:], in_=repm.ap())

            # SBUF accumulator [8, HALF] f32: rows h*4+b, cols = stream pos
            acc = oc.tile([8, HALF], F32, name="acc", tag="acc")

            for k in range(N_PASS):
                # series for this pass: C[32, 2048] (+1 guard col for the
                # t=2047 pair, which is never gathered but is built)
                C = ser.tile([32, N_T + 1], F16, tag="C")
                nc.vector.memset(C[:, N_T:N_T + 1], 0.0)
                nc.sync.dma_start(
                    out=C[:, :N_T],
                    in_=bass.AP(sbt3, k * N_T, [[N_PASS * N_T, 32], [1, N_T]]))

                # pairs table T[128, 2t+e] = C_rep[., t+e] via PE replication
                T = tab.tile([128, 2 * N_T], F16, tag="T")
                c_ap = C[:]
                for q in range(8):
                    t0 = q * 256  # 256 pairs -> 512 psum cols
                    rhs = bass.AP(C.tensor, c_ap.offset + t0,
                                  [c_ap.ap[0], [1, 256], [1, 2]])
                    pp = tps.tile([128, 512], F32, tag="pp")
                    nc.tensor.matmul(out=pp[:], lhsT=rep_tl[:], rhs=rhs,
                                     start=True, stop=True)
                    nc.scalar.copy(out=T[:, q * 512:(q + 1) * 512], in_=pp[:])

                for h in range(2):
                    kh = k * 2 + h
                    I = io.tile([128, S_IDX], I16, tag="I")
                    nc.sync.dma_start(
                        out=I[:],
                        in_=bass.AP(k0w, kh * 128 * S_IDX,
                                    [[S_IDX, 128], [1, S_IDX]]))
                    A = io.tile([128, HALF], U8, tag="A")
                    nc.sync.dma_start(
                        out=A[:],
                        in_=bass.AP(alg, kh * N_G * HALF,
                                    [[HALF, N_G], [0, 16], [1, HALF]]))

                    G = gat.tile([128, HALF * 2], F16, tag="G")
                    nc.gpsimd.ap_gather(
                        out_ap=G[:].rearrange("c (i d) -> c i d", d=2),
                        in_ap=T[:].rearrange("c (e d) -> c e d", d=2),
                        idxs_ap=I[:],
                        channels=128, num_elems=N_T, d=2, num_idxs=HALF)

                    g_ap = G[:]
                    G0 = bass.AP(G.tensor, g_ap.offset, [g_ap.ap[0], [2, HALF]])
                    G1 = bass.AP(G.tensor, g_ap.offset + 1,
                                 [g_ap.ap[0], [2, HALF]])
                    D = lrp.tile([128, HALF], F16, tag="D")
                    nc.vector.tensor_tensor(out=D[:], in0=G1, in1=G0,
                                            op=mybir.AluOpType.subtract)
                    H = lrp.tile([128, HALF], F16, tag="H")
                    nc.vector.tensor_tensor(out=H[:], in0=D[:], in1=A[:],
                                            op=mybir.AluOpType.mult)

                    wa = w2_tl[:, (k * 2 + h) * 16:(k * 2 + h) * 16 + 8]
                    wb = w2_tl[:, (k * 2 + h) * 16 + 8:(k * 2 + h + 1) * 16]
                    for q in range(8):
                        qs = slice(q * 512, (q + 1) * 512)
                        g0q = bass.AP(G.tensor, g_ap.offset + q * 1024,
                                      [g_ap.ap[0], [2, 512]])
                        sc = scp.tile([8, 512], F32, tag="sc")
                        nc.tensor.matmul(out=sc[:], lhsT=wa, rhs=g0q,
                                         start=True, stop=False)
                        nc.tensor.matmul(out=sc[:], lhsT=wb, rhs=H[:, qs],
                                         start=False, stop=True)
                        if k == 0 and h == 0:
                            nc.vector.tensor_copy(out=acc[:, qs], in_=sc[:])
                        else:
                            nc.vector.tensor_tensor(
                                out=acc[:, qs], in0=acc[:, qs], in1=sc[:],
                                op=mybir.AluOpType.add)

            nc.sync.dma_start(out=outd.ap(), in_=acc[:])

    nc.compile()
    return nc


def _host_prep(sino: np.ndarray, lut: np.ndarray):
    sino = np.ascontiguousarray(sino, dtype=np.float32)
    lut = np.ascontiguousarray(lut, dtype=np.float32)
    S16 = sino[:, 0].astype(np.float16)          # [B, D, T]
    lut_flat = lut.reshape(P_TOTAL, N_DET, 2)
    k0_full = np.clip(np.floor(lut_flat[:, :, 0]), 0, N_T - 2).astype(np.int16)
    al_full = np.clip(np.rint(lut_flat[:, :, 1] * 256.0), 0, 255).astype(np.uint8)   # [P, D]

    # sbt3[m=4g+b, k*T+t] = S16[b, g*16+k, t]
    g_idx = np.arange(N_G)
    sbt3 = np.ascontiguousarray(
        S16.transpose(1, 0, 2)                      # [D, B, T]
        .reshape(N_G, 16, B, N_T)                   # [g, k, b, t]
        .transpose(0, 2, 1, 3)                      # [g, b, k, t]
        .reshape(32, N_PASS * N_T))

    # replication matrix rep[m, lane]: lane = 16g + j, m = 4g + (j % 4)
    lanes = np.arange(128)
    m_of_lane = 4 * (lanes // 16) + (lanes % 4)
    repm = np.zeros((32, 128), np.float16)
    repm[m_of_lane, lanes] = 1.0 / 256.0

    apod = (0.5 - 0.5 * np.cos(
        2.0 * np.pi * np.arange(N_DET, dtype=np.float32) / (N_DET - 1)
    )).astype(np.float32)
    norm = max(apod.sum(), np.finfo(np.float32).tiny)
    apod_n = apod / norm

    # w2: per (k,h) 16 cols: first 8 = apod_n*64 (vs table-scaled G0/256),
    # next 8 = apod_n/4 (vs true-scale H = D*alpha_u8/256*256).  /4 averages
    # the 4 replica lanes.
    w2 = np.zeros((128, N_PASS * 32), np.float16)
    for k in range(N_PASS):
        det = (lanes // 16) * 16 + k
        va = (apod_n[det] * 64.0).astype(np.float16)
        vb = (apod_n[det] / 4.0).astype(np.float16)
        for h in range(2):
            base = (k * 2 + h) * 16
            w2[lanes, base + h * 4 + (lanes % 4)] = va
            w2[lanes, base + 8 + h * 4 + (lanes % 4)] = vb

    S_IDX = HALF // 16
    in_maps = []
    for c in range(N_CORES):
        psl = slice(c * PX_PER_CORE, (c + 1) * PX_PER_CORE)
        k0c = k0_full[psl]          # [8192, D]
        alc = al_full[psl]          # [8192, D]
        k0w = np.zeros((N_PASS * 2, 128 * S_IDX), np.int16)
        alg = np.zeros((N_PASS * 2, N_G * HALF), np.uint8)
        for k in range(N_PASS):
            for h in range(2):
                kh = k * 2 + h
                pix = slice(h * HALF, (h + 1) * HALF)
                # idx wrap: lane 16g+j, col s  <- k0[pixel 16s+j, det g*16+k]
                kk = k0c[pix, :]                     # [4096, D]
                aa = alc[pix, :]
                det = g_idx * 16 + k                 # [8]
                kw = kk[:, det]                      # [4096, 8] (i, g)
                # reshape i = 16s+j -> [s, j]; target [g, j, s]
                kw = kw.reshape(S_IDX, 16, N_G).transpose(2, 1, 0)  # [g,j,s]
                k0w[kh] = np.ascontiguousarray(kw).reshape(-1)
                alg[kh] = np.ascontiguousarray(aa[:, det].T).reshape(-1)
        in_maps.append({
            "sbt3": sbt3, "k0w": k0w, "alg": alg, "w2": w2, "repm": repm,
        })
    return in_maps


def _assemble(results: list) -> np.ndarray:
    outs = []
    for r in results:
        o = r["out"]                                 # [8, HALF]
        # rows h*4+b, cols i -> pixel h*HALF+i
        full = np.concatenate([o[0:4], o[4:8]], axis=1)  # [4, 8192]
        outs.append(full)
    allpx = np.concatenate(outs, axis=1)             # [B, P_TOTAL]
    return np.ascontiguousarray(allpx).reshape(B, 1, NY, NX).astype(np.float32)


_CACHE: dict = {}


def _get_nc():
    if "nc" not in _CACHE:
        _CACHE["nc"] = _build_kernel()
    return _CACHE["nc"]


def kernel(sino: np.ndarray, lut: np.ndarray) -> np.ndarray:
    from concourse.bass_utils import run_bass_kernel_spmd

    nc = _get_nc()
    in_maps = _host_prep(np.asarray(sino), np.asarray(lut))
    res = run_bass_kernel_spmd(nc, in_maps, core_ids=list(range(N_CORES)))
    return _assemble(res.results)


def kernel_timed(inputs: dict, iters: int = 20) -> float:
    """Run the kernel repeatedly with device-resident inputs; return ns/iter."""
    import time
    import jax
    from jax.sharding import Mesh, PartitionSpec
    from jax.experimental.shard_map import shard_map
    from concourse.bass2jax import (
        _bass_exec_p, install_neuronx_cc_hook)
    import concourse.mybir as mybir_

    nc = _get_nc()
    in_maps = _host_prep(np.asarray(inputs["sino"]), np.asarray(inputs["lut"]))

    install_neuronx_cc_hook()
    part_name = nc.partition_id_tensor.name if nc.partition_id_tensor else None
    in_names, out_names, out_avals, zero_outs = [], [], [], []
    for alloc in nc.m.functions[0].allocations:
        if not isinstance(alloc, mybir_.MemoryLocationSet):
            continue
        name = alloc.memorylocations[0].name
        if alloc.kind == "ExternalInput":
            if name != part_name:
                in_names.append(name)
        elif alloc.kind == "ExternalOutput":
            out_names.append(name)
            shape = tuple(alloc.tensor_shape)
            dtype = mybir_.dt.np(alloc.dtype)
            out_avals.append(jax.core.ShapedArray(shape, dtype))
            zero_outs.append(np.zeros(shape, dtype))
    n_params = len(in_names)
    all_names = in_names + out_names
    if part_name is not None:
        all_names.append(part_name)
    from concourse.bass2jax import partition_id_tensor

    def _body(*args):
        operands = list(args)
        if part_name is not None:
            operands.append(partition_id_tensor())
        outs = _bass_exec_p.bind(
            *operands,
            out_avals=tuple(out_avals),
            in_names=tuple(all_names),
            out_names=tuple(out_names),
            lowering_input_output_aliases=(),
            sim_require_finite=True,
            sim_require_nnan=True,
            nc=nc,
        )
        return tuple(outs)

    devices = jax.devices()[:N_CORES]
    mesh = Mesh(np.asarray(devices), ("core",))
    n_outs = len(out_names)
    sharded = jax.jit(
        shard_map(_body, mesh=mesh,
                  in_specs=(PartitionSpec("core"),) * (n_params + n_outs),
                  out_specs=(PartitionSpec("core"),) * n_outs,
                  check_rep=False),
        keep_unused=True,
    )
    concat_in = [
        np.concatenate([in_maps[c][name] for c in range(N_CORES)], axis=0)
        for name in in_names
    ]
    concat_zeros = [
        np.zeros((N_CORES * z.shape[0], *z.shape[1:]), z.dtype) for z in zero_outs
    ]
    dev_in = [jax.device_put(a) for a in concat_in]
    dev_zero = [jax.device_put(a) for a in concat_zeros]

    # warmup (compile + 2 runs)
    for _ in range(3):
        outs = sharded(*dev_in, *dev_zero)
        jax.block_until_ready(outs)

    t0 = time.perf_counter()
    for _ in range(iters):
        outs = sharded(*dev_in, *dev_zero)
    jax.block_until_ready(outs)
    t1 = time.perf_counter()
    return (t1 - t0) / iters * 1e9


# revision 25
# speedup vs baseline: 1.0515x; 1.0515x over previous
"""DAS beamforming via GPSIMD ap_gather, u8-quantized alpha.

Layout (per NeuronCore, 8192-pixel shard):
  - 16 passes k; gpsimd core g (partitions 16g..16g+15) handles detector
    d = g*16 + k; lane j within a core carries batch b = j%4 (4 replicas).
  - one full-stream gather per pass: stream position i = pixel i, with the
    ap_gather wrapped-index layout storing k0 for stream position i at
    idx[16g + i%16, i//16] (host prepares this wrap).
  - tables: pairs[c, t, 0:2] = [S(d,t,b)/256, S(d,t+1,b)/256] fp16, built
    per pass by a PE replication matmul (rep^T @ series[32 x 2048] with a
    paired access pattern) since SBUF partitions cannot be broadcast by
    DMA; the 1/256 u8-alpha dequant is folded into rep.
  - ap_gather -> G[128, 8192, 2]; processing then runs per 4096-pixel
    half (PSUM capacity): DVE computes D = G1-G0 and H = D*alpha_u8
    (true scale); PE accumulates psum[8, 512-chunks] += wa^T @ G0_scaled
    + wb^T @ H over all 16 passes, with wa = apod*64 (compensating the
    /256 table scale), wb = apod/4, and zero-padded columns steering
    half 0 into psum rows 0-3 and half 1 into rows 4-7.
"""
import numpy as np

import concourse.bass as bass
import concourse.tile as tile
from concourse import bacc, mybir

N_DET, N_T, NY, NX, B = 128, 2048, 256, 256, 4
P_TOTAL = NY * NX
N_CORES = 8
PX_PER_CORE = P_TOTAL // N_CORES
HALF = PX_PER_CORE // 2            # 4096
N_PASS = 16
N_G = 8                            # gpsimd cores
F32 = mybir.dt.float32
F16 = mybir.dt.float16
I16 = mybir.dt.int16
U8 = mybir.dt.uint8


def _build_kernel():
    nc = bacc.Bacc("TRN2", target_bir_lowering=False, debug=False)

    # sbt3[m = 4g+b, k, t] = sino[b, 0, g*16+k, t]  (fp16 series)
    sbt3 = nc.dram_tensor("sbt3", [32, N_PASS * N_T], F16, kind="ExternalInput")
    # k0w[k, lane, s]: wrapped floor indices (int16), full 8192 stream
    k0w = nc.dram_tensor("k0w", [N_PASS, 128 * (PX_PER_CORE // 16)], I16,
                         kind="ExternalInput")
    # alg[k, g, i] = round(256*alpha[pixel i, det g*16+k]) as u8
    alg = nc.dram_tensor("alg", [N_PASS, N_G * PX_PER_CORE], U8,
                         kind="ExternalInput")
    # w2: per (k,h) 16 cols = 8 wa (apod*64, vs table-scaled G0) then
    # 8 wb (apod/4, vs H); within each 8, only half h's 4 columns are
    # nonzero so the full [8,512] psum write adds 0 on the other half's
    # rows.
    w2 = nc.dram_tensor("w2", [128, N_PASS * 32], F16, kind="ExternalInput")
    # rep[m, lane] = 1/256 if lane's (g,b) == m (replication + dequant)
    repm = nc.dram_tensor("repm", [32, 128], F16, kind="ExternalInput")
    outd = nc.dram_tensor("out", [8, HALF], F32, kind="ExternalOutput")

    S_IDX = PX_PER_CORE // 16  # idx cols (full stream)

    with tile.TileContext(nc) as tc:
        with (
            tc.tile_pool(name="const", bufs=1) as cpool,
            tc.tile_pool(name="ser", bufs=2) as ser,
            tc.tile_pool(name="tab", bufs=2) as tab,
            tc.tile_pool(name="tps", bufs=2, space="PSUM") as tps,
            tc.tile_pool(name="io", bufs=2) as io,
            tc.tile_pool(name="gat", bufs=2) as gat,
            tc.tile_pool(name="lrp", bufs=2) as lrp,
            tc.tile_pool(name="scp", bufs=4, space="PSUM") as scp,
            tc.tile_pool(name="oc", bufs=1) as oc,
        ):
            w2_tl = cpool.tile([128, N_PASS * 32], F16)
            nc.sync.dma_start(out=w2_tl[:], in_=w2.ap())
            rep_tl = cpool.tile([32, 128], F16)
            nc.sync.dma_start(out=rep_tl[:], in_=repm.ap())

            # SBUF accumulator [8, HALF] f32: rows h*4+b, cols = stream pos
            acc = oc.tile([8, HALF], F32, name="acc", tag="acc")

            for k in range(N_PASS):
                # series for this pass: C[32, 2048] (+1 guard col for the
                # t=2047 pair, which is never gathered but is built)
                C = ser.tile([32, N_T + 1], F16, tag="C")
                nc.vector.memset(C[:, N_T:N_T + 1], 0.0)
                nc.sync.dma_start(
                    out=C[:, :N_T],
                    in_=bass.AP(sbt3, k * N_T, [[N_PASS * N_T, 32], [1, N_T]]))

                # pairs table T[128, 2t+e] = C_rep[., t+e] via PE replication
                T = tab.tile([128, 2 * N_T], F16, tag="T")
                c_ap = C[:]
                for q in range(8):
                    t0 = q * 256  # 256 pairs -> 512 psum cols
                    rhs = bass.AP(C.tensor, c_ap.offset + t0,
                                  [c_ap.ap[0], [1, 256], [1, 2]])
                    pp = tps.tile([128, 512], F32, tag="pp")
                    nc.tensor.matmul(out=pp[:], lhsT=rep_tl[:], rhs=rhs,
                                     start=True, stop=True)
                    nc.scalar.copy(out=T[:, q * 512:(q + 1) * 512], in_=pp[:])

                I = io.tile([128, S_IDX], I16, tag="I")
                nc.sync.dma_start(
                    out=I[:],
                    in_=bass.AP(k0w, k * 128 * S_IDX,
                                [[S_IDX, 128], [1, S_IDX]]))
                A = io.tile([128, PX_PER_CORE], U8, tag="A")
                nc.sync.dma_start(
                    out=A[:],
                    in_=bass.AP(alg, k * N_G * PX_PER_CORE,
                                [[PX_PER_CORE, N_G], [0, 16],
                                 [1, PX_PER_CORE]]))

                G = gat.tile([128, PX_PER_CORE * 2], F16, tag="G")
                nc.gpsimd.ap_gather(
                    out_ap=G[:].rearrange("c (i d) -> c i d", d=2),
                    in_ap=T[:].rearrange("c (e d) -> c e d", d=2),
                    idxs_ap=I[:],
                    channels=128, num_elems=N_T, d=2,
                    num_idxs=PX_PER_CORE)
                g_ap = G[:]

                for h in range(2):
                    hoff = h * HALF * 2
                    G0 = bass.AP(G.tensor, g_ap.offset + hoff,
                                 [g_ap.ap[0], [2, HALF]])
                    G1 = bass.AP(G.tensor, g_ap.offset + hoff + 1,
                                 [g_ap.ap[0], [2, HALF]])
                    D = lrp.tile([128, HALF], F16, tag="D")
                    nc.vector.tensor_tensor(out=D[:], in0=G1, in1=G0,
                                            op=mybir.AluOpType.subtract)
                    H = lrp.tile([128, HALF], F16, tag="H")
                    nc.vector.tensor_tensor(out=H[:], in0=D[:],
                                            in1=A[:, h * HALF:(h + 1) * HALF],
                                            op=mybir.AluOpType.mult)

                    wa = w2_tl[:, (k * 2 + h) * 16:(k * 2 + h) * 16 + 8]
                    wb = w2_tl[:, (k * 2 + h) * 16 + 8:(k * 2 + h + 1) * 16]
                    for q in range(8):
                        qs = slice(q * 512, (q + 1) * 512)
                        g0q = bass.AP(G.tensor, g_ap.offset + hoff + q * 1024,
                                      [g_ap.ap[0], [2, 512]])
                        sc = scp.tile([8, 512], F32, tag="sc")
                        nc.tensor.matmul(out=sc[:], lhsT=wa, rhs=g0q,
                                         start=True, stop=False)
                        nc.tensor.matmul(out=sc[:], lhsT=wb, rhs=H[:, qs],
                                         start=False, stop=True)
                        if k == 0 and h == 0:
                            nc.vector.tensor_copy(out=acc[:, qs], in_=sc[:])
                        else:
                            nc.vector.tensor_tensor(
                                out=acc[:, qs], in0=acc[:, qs], in1=sc[:],
                                op=mybir.AluOpType.add)

            nc.sync.dma_start(out=outd.ap(), in_=acc[:])

    nc.compile()
    return nc


def _host_prep(sino: np.ndarray, lut: np.ndarray):
    sino = np.ascontiguousarray(sino, dtype=np.float32)
    lut = np.ascontiguousarray(lut, dtype=np.float32)
    S16 = sino[:, 0].astype(np.float16)          # [B, D, T]
    lut_flat = lut.reshape(P_TOTAL, N_DET, 2)
    k0_full = np.clip(np.floor(lut_flat[:, :, 0]), 0, N_T - 2).astype(np.int16)
    al_full = np.clip(np.rint(lut_flat[:, :, 1] * 256.0), 0, 255).astype(np.uint8)   # [P, D]

    # sbt3[m=4g+b, k*T+t] = S16[b, g*16+k, t]
    g_idx = np.arange(N_G)
    sbt3 = np.ascontiguousarray(
        S16.transpose(1, 0, 2)                      # [D, B, T]
        .reshape(N_G, 16, B, N_T)                   # [g, k, b, t]
        .transpose(0, 2, 1, 3)                      # [g, b, k, t]
        .reshape(32, N_PASS * N_T))

    # replication matrix rep[m, lane]: lane = 16g + j, m = 4g + (j % 4)
    lanes = np.arange(128)
    m_of_lane = 4 * (lanes // 16) + (lanes % 4)
    repm = np.zeros((32, 128), np.float16)
    repm[m_of_lane, lanes] = 1.0 / 256.0

    apod = (0.5 - 0.5 * np.cos(
        2.0 * np.pi * np.arange(N_DET, dtype=np.float32) / (N_DET - 1)
    )).astype(np.float32)
    norm = max(apod.sum(), np.finfo(np.float32).tiny)
    apod_n = apod / norm

    # w2: per (k,h) 16 cols: first 8 = apod_n*64 (vs table-scaled G0/256),
    # next 8 = apod_n/4 (vs true-scale H = D*alpha_u8/256*256).  /4 averages
    # the 4 replica lanes.
    w2 = np.zeros((128, N_PASS * 32), np.float16)
    for k in range(N_PASS):
        det = (lanes // 16) * 16 + k
        va = (apod_n[det] * 64.0).astype(np.float16)
        vb = (apod_n[det] / 4.0).astype(np.float16)
        for h in range(2):
            base = (k * 2 + h) * 16
            w2[lanes, base + h * 4 + (lanes % 4)] = va
            w2[lanes, base + 8 + h * 4 + (lanes % 4)] = vb

    S_IDX = PX_PER_CORE // 16
    in_maps = []
    for c in range(N_CORES):
        psl = slice(c * PX_PER_CORE, (c + 1) * PX_PER_CORE)
        k0c = k0_full[psl]          # [8192, D]
        alc = al_full[psl]          # [8192, D]
        k0w = np.zeros((N_PASS, 128 * S_IDX), np.int16)
        alg = np.zeros((N_PASS, N_G * PX_PER_CORE), np.uint8)
        for k in range(N_PASS):
            det = g_idx * 16 + k                     # [8]
            # idx wrap over the full stream: lane 16g+j, col s
            #   <- k0[pixel 16s+j, det g*16+k]
            kw = k0c[:, det]                         # [8192, 8] (i, g)
            kw = kw.reshape(S_IDX, 16, N_G).transpose(2, 1, 0)  # [g,j,s]
            k0w[k] = np.ascontiguousarray(kw).reshape(-1)
            alg[k] = np.ascontiguousarray(alc[:, det].T).reshape(-1)
        in_maps.append({
            "sbt3": sbt3, "k0w": k0w, "alg": alg, "w2": w2, "repm": repm,
        })
    return in_maps


def _assemble(results: list) -> np.ndarray:
    outs = []
    for r in results:
        o = r["out"]                                 # [8, HALF]
        # rows h*4+b, cols i -> pixel h*HALF+i
        full = np.concatenate([o[0:4], o[4:8]], axis=1)  # [4, 8192]
        outs.append(full)
    allpx = np.concatenate(outs, axis=1)             # [B, P_TOTAL]
    return np.ascontiguousarray(allpx).reshape(B, 1, NY, NX).astype(np.float32)


_CACHE: dict = {}


def _get_nc():
    if "nc" not in _CACHE:
        _CACHE["nc"] = _build_kernel()
    return _CACHE["nc"]


def kernel(sino: np.ndarray, lut: np.ndarray) -> np.ndarray:
    from concourse.bass_utils import run_bass_kernel_spmd

    nc = _get_nc()
    in_maps = _host_prep(np.asarray(sino), np.asarray(lut))
    res = run_bass_kernel_spmd(nc, in_maps, core_ids=list(range(N_CORES)))
    return _assemble(res.results)


def kernel_timed(inputs: dict, iters: int = 20) -> float:
    """Run the kernel repeatedly with device-resident inputs; return ns/iter."""
    import time
    import jax
    from jax.sharding import Mesh, PartitionSpec
    from jax.experimental.shard_map import shard_map
    from concourse.bass2jax import (
        _bass_exec_p, install_neuronx_cc_hook)
    import concourse.mybir as mybir_

    nc = _get_nc()
    in_maps = _host_prep(np.asarray(inputs["sino"]), np.asarray(inputs["lut"]))

    install_neuronx_cc_hook()
    part_name = nc.partition_id_tensor.name if nc.partition_id_tensor else None
    in_names, out_names, out_avals, zero_outs = [], [], [], []
    for alloc in nc.m.functions[0].allocations:
        if not isinstance(alloc, mybir_.MemoryLocationSet):
            continue
        name = alloc.memorylocations[0].name
        if alloc.kind == "ExternalInput":
            if name != part_name:
                in_names.append(name)
        elif alloc.kind == "ExternalOutput":
            out_names.append(name)
            shape = tuple(alloc.tensor_shape)
            dtype = mybir_.dt.np(alloc.dtype)
            out_avals.append(jax.core.ShapedArray(shape, dtype))
            zero_outs.append(np.zeros(shape, dtype))
    n_params = len(in_names)
    all_names = in_names + out_names
    if part_name is not None:
        all_names.append(part_name)
    from concourse.bass2jax import partition_id_tensor

    def _body(*args):
        operands = list(args)
        if part_name is not None:
            operands.append(partition_id_tensor())
        outs = _bass_exec_p.bind(
            *operands,
            out_avals=tuple(out_avals),
            in_names=tuple(all_names),
            out_names=tuple(out_names),
            lowering_input_output_aliases=(),
            sim_require_finite=True,
            sim_require_nnan=True,
            nc=nc,
        )
        return tuple(outs)

    devices = jax.devices()[:N_CORES]
    mesh = Mesh(np.asarray(devices), ("core",))
    n_outs = len(out_names)
    sharded = jax.jit(
        shard_map(_body, mesh=mesh,
                  in_specs=(PartitionSpec("core"),) * (n_params + n_outs),
                  out_specs=(PartitionSpec("core"),) * n_outs,
                  check_rep=False),
        keep_unused=True,
    )
    concat_in = [
        np.concatenate([in_maps[c][name] for c in range(N_CORES)], axis=0)
        for name in in_names
    ]
    concat_zeros = [
        np.zeros((N_CORES * z.shape[0], *z.shape[1:]), z.dtype) for z in zero_outs
    ]
    dev_in = [jax.device_put(a) for a in concat_in]
    dev_zero = [jax.device_put(a) for a in concat_zeros]

    # warmup (compile + 2 runs)
    for _ in range(3):
        outs = sharded(*dev_in, *dev_zero)
        jax.block_until_ready(outs)

    t0 = time.perf_counter()
    for _ in range(iters):
        outs = sharded(*dev_in, *dev_zero)
    jax.block_until_ready(outs)
    t1 = time.perf_counter()
    return (t1 - t0) / iters * 1e9
